# revision 18
# baseline (speedup 1.0000x reference)
"""Trainium2 Bass kernel for CustomTriangleMultiplicationOutgoing.

Reference computation (B=1, N=384, D=C=128):
    z_norm = LN(z) * g + b                        # over D
    left   = (z_norm@Wa + ba) * sigmoid(z_norm@Wga + bga) * mask
    right  = (z_norm@Wb + bb) * sigmoid(z_norm@Wgb + bgb) * mask
    z_out[i,j,c] = sum_k left[i,k,c] * right[j,k,c]
    z_out  = LN(z_out) * g_out + b_out            # over C
    out    = (z_out@Wo + bo) * sigmoid(z_norm@Wgo + bgo)

Host folds the input LN into the weights (LN(z)@W = (z*rstd)@Wcen) and
pre-normalizes z; the device does plain matmuls.

Sharding: phase 1 is row-sharded (48 i-rows per core).  The einsum is
CHANNEL-sharded: an AllToAll (chunked by k for overlap) exchanges
left+right so each core holds all (i,k) for its 16 channels, giving
near-full PE utilization (96-row i-blocks).  A second AllToAll brings
z_out back row-sharded with c on partitions.  Output-LN stats are
accumulated per-channel-shard in phase 2 (bf16 DVE adds) and combined
with a tiny ReduceScatter; rstd is applied to z_out BEFORE the final
projection (it commutes), so phase 3 is a single weight-stationary
streamed matmul wo^T @ zt with the out-gate applied in [d, ij] layout.

Layout trick: the phase-1 gating multiply reads its PSUM pair strided
(r-innermost) and writes CONTIGUOUS pair-interleaved [rb, s, c, r2]
blocks -- strided DVE *writes* cost ~7ns/el while strided reads are
free, so all engine-side transposes are folded into reads and the
A2A staging DMA sees 64-byte runs.
"""

import numpy as np
import ml_dtypes

import concourse.bass as bass
import concourse.mybir as mybir
import concourse.tile as tile
from concourse import bacc
from concourse.bass_utils import run_bass_kernel_spmd

F32 = mybir.dt.float32
BF16 = mybir.dt.bfloat16
EPS = 1e-5

B = 1
N_FULL = 384
D = 128
C = 128
W = 8  # cores
P = 128


def bcast_part(ap, parts):
    """Broadcast a [1, ...] AP across `parts` partitions (partition step 0)."""
    return bass.AP(tensor=ap.tensor, offset=ap.offset, ap=[[0, parts]] + ap.ap[1:])


def build_nc(n=N_FULL, with_bias=False, with_mask=False):
    """Build the SPMD Bass program (same program on all 8 cores)."""
    assert n % P == 0 and n % W == 0
    SH = n // W          # rows of i per core (48)
    KC = n // P          # 128-wide chunks of k (3)
    NT = SH * KC         # 128-row tiles per core (144); tile t=(r,kc): t=r*KC+kc
    CL = C // W          # local channels per core in phase 2 (16)
    CH = CL // 2         # channels per backward-A2A half (8)
    RB = SH // 2         # row pairs (24)
    NIB = KC             # phase-2 i-blocks of 128 rows (3)
    GCH = 512            # go-stream / phase-3 psum chunk columns
    NPOS = SH * n        # positions per core (18432)

    nc = bacc.Bacc(None, num_devices=W)

    zs = nc.declare_dram_parameter("zs", [P, NPOS], BF16, isOutput=False)
    wva = nc.declare_dram_parameter("wva", [D, 2 * C], BF16, isOutput=False)
    wg = nc.declare_dram_parameter("wg", [D, 2 * C], BF16, isOutput=False)
    wgo = nc.declare_dram_parameter("wgo", [D, D], BF16, isOutput=False)
    wo = nc.declare_dram_parameter("wo", [C, D], BF16, isOutput=False)
    if with_bias:
        bva_p = nc.declare_dram_parameter("bva", [1, 2 * C], F32, isOutput=False)
        bg_p = nc.declare_dram_parameter("bg", [1, 2 * C], F32, isOutput=False)
        bgo_p = nc.declare_dram_parameter("bgo", [D, 1], F32, isOutput=False)
        bo_p = nc.declare_dram_parameter("bo", [D, 1], F32, isOutput=False)
    if with_mask:
        mask_sh = nc.declare_dram_parameter("mask_sh", [P, NT], F32, isOutput=False)
    out_d = nc.declare_dram_parameter("out_d", [D, NPOS], F32, isOutput=True)

    # forward A2A, one chunk per kc; block to dest g = [s, k, rb, cl, r2]
    lra2a = [
        nc.dram_tensor(f"lra2a_{kc}", [W, 2, P, RB, CL, 2], BF16) for kc in range(KC)
    ]
    ga2a = [
        nc.dram_tensor(f"ga2a_{kc}", [W, 2, P, RB, CL, 2], BF16) for kc in range(KC)
    ]
    # backward A2A, one chunk per c-half; block to dest g = [c_loc, i_loc, j]
    zoa2a = [nc.dram_tensor(f"zoa2a_{h}", [W, CH, SH, n], BF16) for h in range(2)]
    gza2a = [nc.dram_tensor(f"gza2a_{h}", [W, CH, SH, n], BF16) for h in range(2)]
    # output-LN stats partials + ReduceScatter results
    s_dram = nc.dram_tensor("s_dram", [n, n], BF16)
    sq_dram = nc.dram_tensor("sq_dram", [n, n], BF16)
    s_rs = nc.dram_tensor("s_rs", [SH, n], BF16)
    sq_rs = nc.dram_tensor("sq_rs", [SH, n], BF16)
    rstd_dram = nc.dram_tensor("rstd_dram", [1, NPOS], BF16)

    with tile.TileContext(nc) as tc:
        with tc.tile_pool(name="singles", bufs=1) as singles:
            wva_sb = singles.tile([D, 2 * C], BF16)
            nc.sync.dma_start(wva_sb, wva[:])
            wg_sb = singles.tile([D, 2 * C], BF16)
            nc.sync.dma_start(wg_sb, wg[:])
            wgo_sb = singles.tile([D, D], BF16)
            nc.sync.dma_start(wgo_sb, wgo[:])
            wo_sb = singles.tile([C, D], BF16)
            nc.sync.dma_start(wo_sb, wo[:])
            eps_sb = singles.tile([P, 1], F32)
            nc.vector.memset(eps_sb, EPS)
            if with_bias:
                bva_sb = singles.tile([P, 2 * C], F32)
                nc.sync.dma_start(bva_sb, bcast_part(bva_p[:], P))
                bg_sb = singles.tile([P, 2 * C], F32)
                nc.sync.dma_start(bg_sb, bcast_part(bg_p[:], P))
                bgo_sb = singles.tile([D, 1], F32)
                nc.sync.dma_start(bgo_sb, bgo_p[:])
                bo_sb = singles.tile([D, 1], F32)
                nc.sync.dma_start(bo_sb, bo_p[:])
            if with_mask:
                mask_sb = singles.tile([P, NT], F32)
                nc.sync.dma_start(mask_sb, mask_sh[:])

            # out-gate, [d, ij] layout; even go-chunks hold sigmoid already,
            # odd chunks raw (sigmoid applied in-place during phase 2)
            gg = singles.tile([D, NPOS], BF16)
            gg_v = gg.rearrange("d (r k) -> d r k", r=SH)

            # ---------------- phase 1: projections, kc-major ----------------
            p1pool = tc.alloc_tile_pool(name="p1", bufs=1)
            zs_sb = p1pool.tile([P, NPOS], BF16)
            for ch in range(8):
                w8 = NPOS // 8
                nc.sync.dma_start(
                    zs_sb[:, ch * w8 : (ch + 1) * w8],
                    zs[:, ch * w8 : (ch + 1) * w8],
                )
            zs_v = zs_sb.rearrange("d (r k) -> d r k", r=SH)
            # pair-interleaved gated projections: [k, kc, rb, s(R,L), c, r2]
            lr_buf = p1pool.tile([P, KC, RB, 2, C, 2], BF16)

            odd_go = []  # go-chunks needing their sigmoid in phase 2
            with (
                tc.tile_pool(name="p1_sg", bufs=3) as sgpool,
                tc.tile_pool(name="p1_pv", bufs=2, space="PSUM") as pvpool,
                tc.tile_pool(name="p1_pg", bufs=2, space="PSUM") as pgpool,
                tc.tile_pool(name="p1_go", bufs=2, space="PSUM") as gopool,
            ):
                for kc in range(KC):
                    for rp in range(RB):
                        r0 = 2 * rp
                        ts = [(r0 + j) * KC + kc for j in range(2)]
                        pv = pvpool.tile([P, 2, 2 * C], F32, tag="pv")
                        pg = pgpool.tile([P, 2, 2 * C], F32, tag="pg")
                        for j in range(2):
                            lhsT = zs_v[:, r0 + j, kc * P : (kc + 1) * P]
                            nc.tensor.matmul(
                                pv[:, j, :], lhsT=lhsT, rhs=wva_sb,
                                start=True, stop=True,
                            )
                            nc.tensor.matmul(
                                pg[:, j, :], lhsT=lhsT, rhs=wg_sb,
                                start=True, stop=True,
                            )
                        if with_bias:
                            for j in range(2):
                                nc.vector.tensor_tensor(
                                    pv[:, j, :], pv[:, j, :], bva_sb,
                                    mybir.AluOpType.add,
                                )
                                nc.vector.tensor_tensor(
                                    pg[:, j, :], pg[:, j, :], bg_sb,
                                    mybir.AluOpType.add,
                                )
                        sg = sgpool.tile([P, 2, 2 * C], BF16, tag="sg")
                        nc.scalar.activation(
                            sg, pg, mybir.ActivationFunctionType.Sigmoid
                        )
                        if with_mask:
                            for j in range(2):
                                nc.gpsimd.tensor_scalar_mul(
                                    sg[:, j, :], sg[:, j, :],
                                    mask_sb[:, ts[j] : ts[j] + 1],
                                )
                        # gating: strided psum reads (r2 innermost), contiguous
                        # pair-interleaved write [s, c, r2]
                        nc.vector.tensor_tensor(
                            lr_buf[:, kc, rp],
                            pv.rearrange("k r (s c) -> k s c r", s=2),
                            sg.rearrange("k r (s c) -> k s c r", s=2),
                            mybir.AluOpType.mult,
                        )
                    # out-gate stream for this kc: wgo stationary, zs moving
                    for ch in range(SH // 4):
                        gps = gopool.tile([D, GCH], F32, tag="go")
                        rhs = zs_v[:, 4 * ch : 4 * ch + 4, kc * P : (kc + 1) * P]
                        nc.tensor.matmul(
                            gps, lhsT=wgo_sb, rhs=rhs, start=True, stop=True
                        )
                        if with_bias:
                            nc.vector.tensor_scalar_add(gps, gps, bgo_sb)
                        dst = gg_v[:, 4 * ch : 4 * ch + 4, kc * P : (kc + 1) * P]
                        if (kc * (SH // 4) + ch) % 2 == 0:
                            nc.scalar.activation(
                                dst, gps, mybir.ActivationFunctionType.Sigmoid
                            )
                        else:
                            nc.vector.tensor_copy(dst, gps)
                            odd_go.append((kc, ch))
                    # stage this kc chunk (64B runs) and kick its AllToAll
                    for s in range(2):
                        for g in range(W):
                            nc.sync.dma_start(
                                lra2a[kc][g, s],
                                lr_buf[:, kc, :, s, CL * g : CL * (g + 1), :],
                            )
                    nc.gpsimd.collective_compute(
                        "AllToAll",
                        mybir.AluOpType.bypass,
                        replica_groups=[list(range(W))],
                        ins=[lra2a[kc][:]],
                        outs=[ga2a[kc][:]],
                    )

            p1pool.release()  # zs_sb, lr_buf freed (staged to DRAM)

            # ---------------- phase 2: channel-sharded einsum ----------------
            p2big = tc.alloc_tile_pool(name="p2big", bufs=1)
            # z_out staging [i(128) x cl x j] + bf16 stats accumulators
            zo_sb = [
                p2big.tile([P, CL, n], BF16, name=f"zo_sb{ib}") for ib in range(NIB)
            ]
            s_acc = [
                p2big.tile([P, n], BF16, name=f"s_acc{ib}") for ib in range(NIB)
            ]
            sq_acc = [
                p2big.tile([P, n], BF16, name=f"sq_acc{ib}") for ib in range(NIB)
            ]

            lr_all = tc.alloc_tile_pool(name="lr_all", bufs=1)
            L_all = [
                lr_all.tile([P, W, RB, CL, 2], BF16, name=f"L_all{kc}")
                for kc in range(KC)
            ]
            R_all = [
                lr_all.tile([P, W, RB, CL, 2], BF16, name=f"R_all{kc}")
                for kc in range(KC)
            ]
            for kc in range(KC):
                nc.sync.dma_start(
                    L_all[kc],
                    ga2a[kc][:, 1].rearrange("g k rb cl r -> k g rb cl r"),
                )
                nc.sync.dma_start(
                    R_all[kc],
                    ga2a[kc][:, 0].rearrange("g k rb cl r -> k g rb cl r"),
                )

            with (
                tc.tile_pool(name="p2_lc", bufs=3) as lcpool,
                tc.tile_pool(name="p2_sq", bufs=3) as sqpool,
                tc.tile_pool(name="p2_ps", bufs=6, space="PSUM") as p2psum,
            ):
                # finish the out-gate sigmoids on the otherwise-idle ACT
                for kc, ch in odd_go:
                    dst = gg_v[:, 4 * ch : 4 * ch + 4, kc * P : (kc + 1) * P]
                    nc.scalar.activation(
                        dst, dst, mybir.ActivationFunctionType.Sigmoid
                    )
                for cl in range(CL):
                    # weights APs allow only ONE free dim: gather this
                    # channel's left operand contiguously (strided DVE
                    # reads are free, writes contiguous)
                    lc = lcpool.tile([P, KC, n], BF16, tag="lc")
                    for kc in range(KC):
                        nc.vector.tensor_copy(
                            lc[:, kc].rearrange("k (s rb r) -> k s rb r", s=W, r=2),
                            L_all[kc][:, :, :, cl, :],
                        )
                    for ib in range(NIB):
                        ps = p2psum.tile([P, n], F32, tag="p2")
                        for kc in range(KC):
                            nc.tensor.matmul(
                                ps,
                                lhsT=lc[:, kc, ib * P : (ib + 1) * P],
                                rhs=R_all[kc][:, :, :, cl, :],
                                start=(kc == 0),
                                stop=(kc == KC - 1),
                            )
                        zslc = zo_sb[ib][:, cl, :]
                        if (cl * NIB + ib) % 2 == 0:
                            nc.vector.tensor_copy(zslc, ps)
                        else:
                            nc.scalar.copy(zslc, ps)
                        # bf16 stats partials (4x DVE mode: all-SBUF 2-byte)
                        sqt = sqpool.tile([P, n], BF16, tag="sqt")
                        nc.vector.tensor_tensor(
                            sqt, zslc, zslc, mybir.AluOpType.mult
                        )
                        if cl == 0:
                            nc.vector.tensor_copy(s_acc[ib], zslc)
                            nc.vector.tensor_copy(sq_acc[ib], sqt)
                        else:
                            eng = nc.vector if ib != 1 else nc.gpsimd
                            eng.tensor_tensor(
                                s_acc[ib], s_acc[ib], zslc, mybir.AluOpType.add
                            )
                            eng.tensor_tensor(
                                sq_acc[ib], sq_acc[ib], sqt, mybir.AluOpType.add
                            )
                    # after each c-half: stage + backward AllToAll
                    if cl == CH - 1 or cl == CL - 1:
                        h = 0 if cl < CH else 1
                        c0 = h * CH
                        for g in range(W):
                            lo, hi = g * SH, (g + 1) * SH
                            while lo < hi:
                                ib_ = lo // P
                                seg = min(hi, (ib_ + 1) * P) - lo
                                nc.sync.dma_start(
                                    zoa2a[h][g]
                                    .rearrange("c i j -> i c j")[
                                        lo - g * SH : lo - g * SH + seg
                                    ],
                                    zo_sb[ib_][lo - ib_ * P : lo - ib_ * P + seg,
                                               c0 : c0 + CH, :],
                                )
                                lo += seg
                        nc.gpsimd.collective_compute(
                            "AllToAll",
                            mybir.AluOpType.bypass,
                            replica_groups=[list(range(W))],
                            ins=[zoa2a[h][:]],
                            outs=[gza2a[h][:]],
                        )
                # stats: stage partials, ReduceScatter-add across cores
                for ib in range(NIB):
                    nc.sync.dma_start(s_dram[ib * P : (ib + 1) * P, :], s_acc[ib])
                    nc.sync.dma_start(
                        sq_dram[ib * P : (ib + 1) * P, :], sq_acc[ib]
                    )
                nc.gpsimd.collective_compute(
                    "ReduceScatter", mybir.AluOpType.add,
                    replica_groups=[list(range(W))],
                    ins=[s_dram[:]], outs=[s_rs[:]],
                )
                nc.gpsimd.collective_compute(
                    "ReduceScatter", mybir.AluOpType.add,
                    replica_groups=[list(range(W))],
                    ins=[sq_dram[:]], outs=[sq_rs[:]],
                )

            lr_all.release()

            # ---------------- phase 3: rstd, z_out @ Wo, gate ----------------
            p3big = tc.alloc_tile_pool(name="p3big", bufs=1)
            zt = p3big.tile([C, NPOS], BF16)  # z_out, c on partitions
            for h in range(2):
                for src in range(W):
                    nc.sync.dma_start(
                        zt[CL * src + CH * h : CL * src + CH * (h + 1), :],
                        gza2a[h][src].rearrange("c i j -> c (i j)"),
                    )
            # rstd = 1/sqrt(E[x^2] - E[x]^2 + eps), computed on [48, n]
            s_sb = p3big.tile([SH, n], BF16)
            nc.sync.dma_start(s_sb, s_rs[:])
            sq_sb = p3big.tile([SH, n], BF16)
            nc.sync.dma_start(sq_sb, sq_rs[:])
            mean = p3big.tile([SH, n], F32)
            nc.vector.tensor_scalar_mul(mean, s_sb, 1.0 / C)
            msq = p3big.tile([SH, n], F32)
            nc.vector.tensor_scalar_mul(msq, sq_sb, 1.0 / C)
            var = p3big.tile([SH, n], F32)
            nc.vector.tensor_tensor(var, mean, mean, mybir.AluOpType.mult)
            nc.vector.tensor_tensor(var, msq, var, mybir.AluOpType.subtract)
            rstd = p3big.tile([SH, n], F32)
            nc.scalar.activation(
                rstd, var, mybir.ActivationFunctionType.Sqrt, bias=eps_sb[0:SH]
            )
            nc.vector.reciprocal(rstd, rstd)
            rstd_bf = p3big.tile([SH, n], BF16)
            nc.vector.tensor_copy(rstd_bf, rstd)
            # roundtrip through DRAM so rstd can be broadcast across
            # partitions by replicated DMA reads (0-step partition APs are
            # DMA-only)
            nc.sync.dma_start(rstd_dram[:].rearrange("o (i j) -> (o i) j", i=SH),
                              rstd_bf)

            NCH = NPOS // GCH  # 36 chunks
            RBC = 4 * GCH      # rstd broadcast chunk columns
            with (
                tc.tile_pool(name="p3_rb", bufs=2) as rbpool,
                tc.tile_pool(name="p3_ot", bufs=2) as otpool,
                tc.tile_pool(name="p3_ps", bufs=4, space="PSUM") as p3psum,
            ):
                # scale zt by rstd in place (4x DVE), then stream wo^T @ zt
                for bc in range(NPOS // RBC):
                    sl = slice(bc * RBC, (bc + 1) * RBC)
                    rb_t = rbpool.tile([C, RBC], BF16, tag="rb")
                    nc.sync.dma_start(rb_t, bcast_part(rstd_dram[:, sl], C))
                    nc.vector.tensor_tensor(
                        zt[:, sl], zt[:, sl], rb_t, mybir.AluOpType.mult
                    )
                OCH = 4  # psum chunks per output DMA
                for og in range(NCH // OCH):
                    ot = otpool.tile([D, OCH, GCH], F32, tag="ot")
                    for oc in range(OCH):
                        ch = og * OCH + oc
                        sl = slice(ch * GCH, (ch + 1) * GCH)
                        pr = p3psum.tile([D, GCH], F32, tag="pr")
                        nc.tensor.matmul(
                            pr, lhsT=wo_sb, rhs=zt[:, sl], start=True, stop=True
                        )
                        if with_bias:
                            nc.vector.tensor_scalar_add(pr, pr, bo_sb)
                        eng = nc.vector if oc % 2 == 0 else nc.scalar
                        if eng is nc.vector:
                            nc.vector.tensor_tensor(
                                ot[:, oc, :], pr, gg[:, sl], mybir.AluOpType.mult
                            )
                        else:
                            # keep ACT useful: evacuate with scale=1 then Pool
                            # multiplies the gate in SBUF
                            po = otpool.tile([D, GCH], BF16, tag="po")
                            nc.scalar.copy(po, pr)
                            nc.gpsimd.tensor_tensor(
                                ot[:, oc, :], po, gg[:, sl], mybir.AluOpType.mult
                            )
                    nc.sync.dma_start(
                        out_d[:, og * OCH * GCH : (og + 1) * OCH * GCH],
                        ot,
                    )
            p3big.release()
            p2big.release()

    nc.compile()
    return nc


_CACHE = {}


def _get_nc(n, with_bias, with_mask):
    key = (n, with_bias, with_mask)
    if key not in _CACHE:
        _CACHE[key] = build_nc(n=n, with_bias=with_bias, with_mask=with_mask)
    return _CACHE[key]


def prepare_host(z, mask, norm_g, norm_b, norm_out_g, norm_out_b,
                 Wa, ba, Wb, bb, Wga, bga, Wgb, bgb, Wo, bo, Wgo, bgo, n=N_FULL):
    """Fold norm affines + centering into weights; pre-normalize z rows."""
    f = np.asarray
    z = f(z, dtype=np.float32)
    mask = f(mask, dtype=np.float32)
    g = f(norm_g, np.float32)
    b = f(norm_b, np.float32)
    go = f(norm_out_g, np.float32)
    bo_n = f(norm_out_b, np.float32)

    # LN(z) @ W_aff + bias = (z*rstd) @ Wcen + (b @ W + bias),
    # Wcen = (I - J/D)(g .* W)
    def fold(Wm, bias):
        Wm = f(Wm, np.float32)
        Wg = g[:, None] * Wm
        Wcen = Wg - np.mean(Wg, axis=0, keepdims=True)
        return Wcen, f(bias, np.float32) + b @ Wm

    Wa_, ba_ = fold(Wa, ba)
    Wga_, bga_ = fold(Wga, bga)
    Wb_, bb_ = fold(Wb, bb)
    Wgb_, bgb_ = fold(Wgb, bgb)
    Wgo_, bgo_ = fold(Wgo, bgo)
    Wo32 = f(Wo, np.float32)
    Wog = go[:, None] * Wo32
    Wo_ = Wog - np.mean(Wog, axis=0, keepdims=True)
    bo_ = f(bo, np.float32) + bo_n @ Wo32

    bf = ml_dtypes.bfloat16
    wva_h = np.concatenate([Wb_, Wa_], axis=1).astype(bf)
    wg_h = np.concatenate([Wgb_, Wga_], axis=1).astype(bf)
    wgo_h = Wgo_.astype(bf)
    wo_h = Wo_.astype(bf)
    bva_h = np.concatenate([bb_, ba_])[None, :].astype(np.float32)
    bg_h = np.concatenate([bgb_, bga_])[None, :].astype(np.float32)

    with_bias = bool(
        np.any(bva_h) or np.any(bg_h) or np.any(bgo_) or np.any(bo_)
    )
    with_mask = not bool(np.all(mask == 1.0))

    # host-side LN stats: rstd per row of z, folded into z itself
    zf = z[0].reshape(n * n, D)
    m = zf.mean(axis=1, keepdims=True)
    v = ((zf - m) ** 2).mean(axis=1, keepdims=True)
    r = 1.0 / np.sqrt(v + EPS)
    zsf = (zf * r).astype(np.float32)

    SH = n // W
    NT = SH * n // P
    in_maps = []
    for mi in range(W):
        rows = zsf[SH * n * mi : SH * n * (mi + 1)]  # [SH*n, D]
        zs_h = np.ascontiguousarray(rows.T).astype(bf)  # [D, SH*n]
        im = {
            "zs": zs_h,
            "wva": wva_h,
            "wg": wg_h,
            "wgo": wgo_h,
            "wo": wo_h,
        }
        if with_bias:
            im["bva"] = bva_h
            im["bg"] = bg_h
            im["bgo"] = bgo_[:, None].astype(np.float32)
            im["bo"] = bo_[:, None].astype(np.float32)
        if with_mask:
            msk = mask[0].reshape(n * n)[SH * n * mi : SH * n * (mi + 1)]
            im["mask_sh"] = np.ascontiguousarray(
                msk.reshape(NT, P).T
            ).astype(np.float32)
        in_maps.append(im)
    return in_maps, with_bias, with_mask


def unshard(results, n=N_FULL):
    """results: list of per-core out_d arrays [D, SH*n] -> [1, n, n, D]."""
    SH = n // W
    parts = []
    for mi in range(W):
        o = results[mi].reshape(D, SH, n)
        parts.append(o.transpose(1, 2, 0))
    return np.concatenate(parts, axis=0)[None]


def kernel(**inputs):
    n = inputs["z"].shape[1]
    in_maps, with_bias, with_mask = prepare_host(**inputs, n=n)
    nc = _get_nc(n, with_bias, with_mask)
    res = run_bass_kernel_spmd(nc, in_maps, list(range(W)))
    out = unshard([res.results[m]["out_d"] for m in range(W)], n=n)
    return out.astype(np.float32)


# revision 21
# speedup vs baseline: 1.0346x; 1.0346x over previous
"""Trainium2 Bass kernel for CustomTriangleMultiplicationOutgoing.

Reference computation (B=1, N=384, D=C=128):
    z_norm = LN(z) * g + b                        # over D
    left   = (z_norm@Wa + ba) * sigmoid(z_norm@Wga + bga) * mask
    right  = (z_norm@Wb + bb) * sigmoid(z_norm@Wgb + bgb) * mask
    z_out[i,j,c] = sum_k left[i,k,c] * right[j,k,c]
    z_out  = LN(z_out) * g_out + b_out            # over C
    out    = (z_out@Wo + bo) * sigmoid(z_norm@Wgo + bgo)

Host folds the input LN into the weights (LN(z)@W = (z*rstd)@Wcen) and
pre-normalizes z; the device does plain matmuls.

Sharding: phase 1 is row-sharded (48 i-rows per core).  The einsum is
CHANNEL-sharded: an AllToAll (chunked by k for overlap) exchanges
left+right so each core holds all (i,k) for its 16 channels, giving
near-full PE utilization (96-row i-blocks).  A second AllToAll brings
z_out back row-sharded with c on partitions.  Output-LN stats are
accumulated per-channel-shard in phase 2 (bf16 DVE adds) and combined
with a tiny ReduceScatter; rstd is applied to z_out BEFORE the final
projection (it commutes), so phase 3 is a single weight-stationary
streamed matmul wo^T @ zt with the out-gate applied in [d, ij] layout.

Layout trick: the phase-1 gating multiply reads its PSUM pair strided
(r-innermost) and writes CONTIGUOUS pair-interleaved [rb, s, c, r2]
blocks -- strided DVE *writes* cost ~7ns/el while strided reads are
free, so all engine-side transposes are folded into reads and the
A2A staging DMA sees 64-byte runs.
"""

import numpy as np
import ml_dtypes

import concourse.bass as bass
import concourse.mybir as mybir
import concourse.tile as tile
from concourse import bacc
from concourse.masks import make_identity
from concourse.bass_utils import run_bass_kernel_spmd

F32 = mybir.dt.float32
BF16 = mybir.dt.bfloat16
EPS = 1e-5

B = 1
N_FULL = 384
D = 128
C = 128
W = 8  # cores
P = 128


def bcast_part(ap, parts):
    """Broadcast a [1, ...] AP across `parts` partitions (partition step 0)."""
    return bass.AP(tensor=ap.tensor, offset=ap.offset, ap=[[0, parts]] + ap.ap[1:])


def build_nc(n=N_FULL, with_bias=False, with_mask=False):
    """Build the SPMD Bass program (same program on all 8 cores)."""
    assert n % P == 0 and n % W == 0
    SH = n // W          # rows of i per core (48)
    KC = n // P          # 128-wide chunks of k (3)
    NT = SH * KC         # 128-row tiles per core (144); tile t=(r,kc): t=r*KC+kc
    CL = C // W          # local channels per core in phase 2 (16)
    CH = CL // 2         # channels per backward-A2A half (8)
    RB = SH // 2         # row pairs (24)
    NIB = KC             # phase-2 i-blocks of 128 rows (3)
    GCH = 512            # go-stream / phase-3 psum chunk columns
    NPOS = SH * n        # positions per core (18432)

    nc = bacc.Bacc(None, num_devices=W)

    zs = nc.declare_dram_parameter("zs", [P, NPOS], BF16, isOutput=False)
    wva = nc.declare_dram_parameter("wva", [D, 2 * C], BF16, isOutput=False)
    wg = nc.declare_dram_parameter("wg", [D, 2 * C], BF16, isOutput=False)
    wgo = nc.declare_dram_parameter("wgo", [D, D], BF16, isOutput=False)
    wo = nc.declare_dram_parameter("wo", [C, D], BF16, isOutput=False)
    if with_bias:
        bva_p = nc.declare_dram_parameter("bva", [1, 2 * C], F32, isOutput=False)
        bg_p = nc.declare_dram_parameter("bg", [1, 2 * C], F32, isOutput=False)
        bgo_p = nc.declare_dram_parameter("bgo", [D, 1], F32, isOutput=False)
        bo_p = nc.declare_dram_parameter("bo", [D, 1], F32, isOutput=False)
    if with_mask:
        mask_sh = nc.declare_dram_parameter("mask_sh", [P, NT], F32, isOutput=False)
    out_d = nc.declare_dram_parameter("out_d", [D, NPOS], F32, isOutput=True)

    # forward A2A, one chunk per kc; block to dest g = [s, k, rb, cl, r2]
    lra2a = [
        nc.dram_tensor(f"lra2a_{kc}", [W, 2, P, RB, CL, 2], BF16) for kc in range(KC)
    ]
    ga2a = [
        nc.dram_tensor(f"ga2a_{kc}", [W, 2, P, RB, CL, 2], BF16) for kc in range(KC)
    ]
    # backward A2A, one chunk per c-half; block to dest g = [c_loc, i_loc, j]
    zoa2a = [nc.dram_tensor(f"zoa2a_{h}", [W, CH, SH, n], BF16) for h in range(2)]
    gza2a = [nc.dram_tensor(f"gza2a_{h}", [W, CH, SH, n], BF16) for h in range(2)]
    rstd_dram = nc.dram_tensor("rstd_dram", [1, NPOS], BF16)

    with tile.TileContext(nc) as tc:
        with tc.tile_pool(name="singles", bufs=1) as singles:
            wva_sb = singles.tile([D, 2 * C], BF16)
            nc.sync.dma_start(wva_sb, wva[:])
            wg_sb = singles.tile([D, 2 * C], BF16)
            nc.sync.dma_start(wg_sb, wg[:])
            wgo_sb = singles.tile([D, D], BF16)
            nc.sync.dma_start(wgo_sb, wgo[:])
            wo_sb = singles.tile([C, D], BF16)
            nc.sync.dma_start(wo_sb, wo[:])
            eps_sb = singles.tile([P, 1], F32)
            nc.vector.memset(eps_sb, EPS)
            ones_bf = singles.tile([P, 1], BF16)
            nc.vector.memset(ones_bf, 1.0)
            ident = singles.tile([P, P], BF16)
            make_identity(nc, ident)
            if with_bias:
                bva_sb = singles.tile([P, 2 * C], F32)
                nc.sync.dma_start(bva_sb, bcast_part(bva_p[:], P))
                bg_sb = singles.tile([P, 2 * C], F32)
                nc.sync.dma_start(bg_sb, bcast_part(bg_p[:], P))
                bgo_sb = singles.tile([D, 1], F32)
                nc.sync.dma_start(bgo_sb, bgo_p[:])
                bo_sb = singles.tile([D, 1], F32)
                nc.sync.dma_start(bo_sb, bo_p[:])
            if with_mask:
                mask_sb = singles.tile([P, NT], F32)
                nc.sync.dma_start(mask_sb, mask_sh[:])

            # out-gate, [d, ij] layout; even go-chunks hold sigmoid already,
            # odd chunks raw (sigmoid applied in-place during phase 2)
            gg = singles.tile([D, NPOS], BF16)
            gg_v = gg.rearrange("d (r k) -> d r k", r=SH)

            # ---------------- phase 1: projections, kc-major ----------------
            p1pool = tc.alloc_tile_pool(name="p1", bufs=1)
            zs_sb = p1pool.tile([P, NPOS], BF16)
            for ch in range(8):
                w8 = NPOS // 8
                nc.sync.dma_start(
                    zs_sb[:, ch * w8 : (ch + 1) * w8],
                    zs[:, ch * w8 : (ch + 1) * w8],
                )
            zs_v = zs_sb.rearrange("d (r k) -> d r k", r=SH)
            # pair-interleaved gated projections: [k, kc, rb, s(R,L), c, r2]
            lr_buf = p1pool.tile([P, KC, RB, 2, C, 2], BF16)

            odd_go = []  # go-chunks needing their sigmoid in phase 2
            with (
                tc.tile_pool(name="p1_sg", bufs=3) as sgpool,
                tc.tile_pool(name="p1_pv", bufs=2, space="PSUM") as pvpool,
                tc.tile_pool(name="p1_pg", bufs=2, space="PSUM") as pgpool,
                tc.tile_pool(name="p1_go", bufs=2, space="PSUM") as gopool,
            ):
                for kc in range(KC):
                    for rp in range(RB):
                        r0 = 2 * rp
                        ts = [(r0 + j) * KC + kc for j in range(2)]
                        pv = pvpool.tile([P, 2, 2 * C], F32, tag="pv")
                        pg = pgpool.tile([P, 2, 2 * C], F32, tag="pg")
                        for j in range(2):
                            lhsT = zs_v[:, r0 + j, kc * P : (kc + 1) * P]
                            nc.tensor.matmul(
                                pv[:, j, :], lhsT=lhsT, rhs=wva_sb,
                                start=True, stop=True,
                            )
                            nc.tensor.matmul(
                                pg[:, j, :], lhsT=lhsT, rhs=wg_sb,
                                start=True, stop=True,
                            )
                        if with_bias:
                            for j in range(2):
                                nc.vector.tensor_tensor(
                                    pv[:, j, :], pv[:, j, :], bva_sb,
                                    mybir.AluOpType.add,
                                )
                                nc.vector.tensor_tensor(
                                    pg[:, j, :], pg[:, j, :], bg_sb,
                                    mybir.AluOpType.add,
                                )
                        sg = sgpool.tile([P, 2, 2 * C], BF16, tag="sg")
                        nc.scalar.activation(
                            sg, pg, mybir.ActivationFunctionType.Sigmoid
                        )
                        if with_mask:
                            for j in range(2):
                                nc.gpsimd.tensor_scalar_mul(
                                    sg[:, j, :], sg[:, j, :],
                                    mask_sb[:, ts[j] : ts[j] + 1],
                                )
                        # gating: strided psum reads (r2 innermost), contiguous
                        # pair-interleaved write [s, c, r2]
                        nc.vector.tensor_tensor(
                            lr_buf[:, kc, rp],
                            pv.rearrange("k r (s c) -> k s c r", s=2),
                            sg.rearrange("k r (s c) -> k s c r", s=2),
                            mybir.AluOpType.mult,
                        )
                    # stage this kc chunk (64B runs) and kick its AllToAll
                    for s in range(2):
                        for g in range(W):
                            eng = nc.sync if g % 2 == 0 else nc.scalar
                            eng.dma_start(
                                lra2a[kc][g, s],
                                lr_buf[:, kc, :, s, CL * g : CL * (g + 1), :],
                            )
                    nc.gpsimd.collective_compute(
                        "AllToAll",
                        mybir.AluOpType.bypass,
                        replica_groups=[list(range(W))],
                        ins=[lra2a[kc][:]],
                        outs=[ga2a[kc][:]],
                    )
                    # out-gate stream for this kc: wgo stationary, zs moving
                    for ch in range(SH // 4):
                        gps = gopool.tile([D, GCH], F32, tag="go")
                        rhs = zs_v[:, 4 * ch : 4 * ch + 4, kc * P : (kc + 1) * P]
                        nc.tensor.matmul(
                            gps, lhsT=wgo_sb, rhs=rhs, start=True, stop=True
                        )
                        if with_bias:
                            nc.vector.tensor_scalar_add(gps, gps, bgo_sb)
                        dst = gg_v[:, 4 * ch : 4 * ch + 4, kc * P : (kc + 1) * P]
                        if (kc * (SH // 4) + ch) % 2 == 0:
                            nc.scalar.activation(
                                dst, gps, mybir.ActivationFunctionType.Sigmoid
                            )
                        else:
                            nc.vector.tensor_copy(dst, gps)
                            odd_go.append((kc, ch))

            p1pool.release()  # zs_sb, lr_buf freed (staged to DRAM)

            # ---------------- phase 2: channel-sharded einsum ----------------
            p2big = tc.alloc_tile_pool(name="p2big", bufs=1)
            # z_out staging [i(128) x cl x j] + bf16 stats accumulators
            zo_sb = [
                p2big.tile([P, CL, n], BF16, name=f"zo_sb{ib}") for ib in range(NIB)
            ]

            lr_all = tc.alloc_tile_pool(name="lr_all", bufs=1)
            L_all = [
                lr_all.tile([P, W, RB, CL, 2], BF16, name=f"L_all{kc}")
                for kc in range(KC)
            ]
            R_all = [
                lr_all.tile([P, W, RB, CL, 2], BF16, name=f"R_all{kc}")
                for kc in range(KC)
            ]
            for kc in range(KC):
                nc.sync.dma_start(
                    L_all[kc],
                    ga2a[kc][:, 1].rearrange("g k rb cl r -> k g rb cl r"),
                )
                nc.sync.dma_start(
                    R_all[kc],
                    ga2a[kc][:, 0].rearrange("g k rb cl r -> k g rb cl r"),
                )

            with (
                tc.tile_pool(name="p2_lc", bufs=3) as lcpool,
                tc.tile_pool(name="p2_sq", bufs=3) as sqpool,
                tc.tile_pool(name="p2_ps", bufs=6, space="PSUM") as p2psum,
            ):
                # finish the out-gate sigmoids on the otherwise-idle ACT
                for kc, ch in odd_go:
                    dst = gg_v[:, 4 * ch : 4 * ch + 4, kc * P : (kc + 1) * P]
                    nc.scalar.activation(
                        dst, dst, mybir.ActivationFunctionType.Sigmoid
                    )
                for cl in range(CL):
                    # weights APs allow only ONE free dim: gather this
                    # channel's left operand contiguously (strided DVE
                    # reads are free, writes contiguous)
                    lc = lcpool.tile([P, KC, n], BF16, tag="lc")
                    for kc in range(KC):
                        eng = nc.vector if (cl * KC + kc) % 3 else nc.gpsimd
                        eng.tensor_copy(
                            lc[:, kc].rearrange("k (s rb r) -> k s rb r", s=W, r=2),
                            L_all[kc][:, :, :, cl, :],
                        )
                    for ib in range(NIB):
                        ps = p2psum.tile([P, n], F32, tag="p2")
                        for kc in range(KC):
                            nc.tensor.matmul(
                                ps,
                                lhsT=lc[:, kc, ib * P : (ib + 1) * P],
                                rhs=R_all[kc][:, :, :, cl, :],
                                start=(kc == 0),
                                stop=(kc == KC - 1),
                            )
                        zslc = zo_sb[ib][:, cl, :]
                        if (cl * NIB + ib) % 2 == 0:
                            nc.vector.tensor_copy(zslc, ps)
                        else:
                            nc.scalar.copy(zslc, ps)
                    # after each c-half: stage + backward AllToAll
                    if cl == CH - 1 or cl == CL - 1:
                        h = 0 if cl < CH else 1
                        c0 = h * CH
                        for g in range(W):
                            lo, hi = g * SH, (g + 1) * SH
                            while lo < hi:
                                ib_ = lo // P
                                seg = min(hi, (ib_ + 1) * P) - lo
                                nc.sync.dma_start(
                                    zoa2a[h][g]
                                    .rearrange("c i j -> i c j")[
                                        lo - g * SH : lo - g * SH + seg
                                    ],
                                    zo_sb[ib_][lo - ib_ * P : lo - ib_ * P + seg,
                                               c0 : c0 + CH, :],
                                )
                                lo += seg
                        nc.gpsimd.collective_compute(
                            "AllToAll",
                            mybir.AluOpType.bypass,
                            replica_groups=[list(range(W))],
                            ins=[zoa2a[h][:]],
                            outs=[gza2a[h][:]],
                        )

            lr_all.release()

            # ---------------- phase 3: rstd, z_out @ Wo, gate ----------------
            p3big = tc.alloc_tile_pool(name="p3big", bufs=1)
            zt = p3big.tile([C, NPOS], BF16)  # z_out, c on partitions
            for h in range(2):
                for src in range(W):
                    nc.sync.dma_start(
                        zt[CL * src + CH * h : CL * src + CH * (h + 1), :],
                        gza2a[h][src].rearrange("c i j -> c (i j)"),
                    )
            # output-LN stats on the receiver: per-tile 1-column matmuls
            # against a ones vector contract over c (the partition dim)
            stpool = tc.alloc_tile_pool(name="p3_st", bufs=1, space="PSUM")
            S_ps = stpool.tile([P, NT], F32)
            SQ_ps = stpool.tile([P, NT], F32)
            with tc.tile_pool(name="p3_sq", bufs=3) as sq3pool:
                for qg in range(NT // 4):
                    sqq = sq3pool.tile([P, 4 * P], BF16, tag="sqq")
                    zq = zt[:, 4 * P * qg : 4 * P * (qg + 1)]
                    nc.vector.tensor_tensor(sqq, zq, zq, mybir.AluOpType.mult)
                    for tl in range(4):
                        t3 = 4 * qg + tl
                        nc.tensor.matmul(
                            S_ps[:, t3 : t3 + 1],
                            lhsT=zt[:, t3 * P : (t3 + 1) * P],
                            rhs=ones_bf, start=True, stop=True,
                        )
                        nc.tensor.matmul(
                            SQ_ps[:, t3 : t3 + 1],
                            lhsT=sqq[:, tl * P : (tl + 1) * P],
                            rhs=ones_bf, start=True, stop=True,
                        )
            # rstd in [p, t3] tile-major
            mean = p3big.tile([P, NT], F32)
            nc.vector.tensor_scalar_mul(mean, S_ps, 1.0 / C)
            msq = p3big.tile([P, NT], F32)
            nc.vector.tensor_scalar_mul(msq, SQ_ps, 1.0 / C)
            var = p3big.tile([P, NT], F32)
            nc.vector.tensor_tensor(var, mean, mean, mybir.AluOpType.mult)
            nc.vector.tensor_tensor(var, msq, var, mybir.AluOpType.subtract)
            rstd = p3big.tile([P, NT], F32)
            nc.scalar.activation(
                rstd, var, mybir.ActivationFunctionType.Sqrt, bias=eps_sb
            )
            nc.vector.reciprocal(rstd, rstd)
            rstd_bf = p3big.tile([P, NT], BF16)
            nc.vector.tensor_copy(rstd_bf, rstd)
            stpool.release()
            # transpose [p, t3] -> position-major row in DRAM (PE transpose,
            # then contiguous DMA; replicated DMA reads broadcast it back)
            with tc.tile_pool(name="p3_tp", bufs=2, space="PSUM") as tppool:
                tp0 = tppool.tile([P, P], BF16, tag="tp0")
                nc.tensor.transpose(tp0, rstd_bf[:, 0:P], ident)
                rT0 = p3big.tile([P, P], BF16)
                nc.vector.tensor_copy(rT0, tp0)
                nc.sync.dma_start(
                    rstd_dram[:, 0 : P * P].rearrange("o (t p) -> (o t) p", t=P),
                    rT0,
                )
                tp1 = tppool.tile([NT - P, P], BF16, tag="tp1")
                nc.tensor.transpose(tp1, rstd_bf[:, P:NT], ident)
                rT1 = p3big.tile([NT - P, P], BF16)
                nc.vector.tensor_copy(rT1, tp1)
                nc.sync.dma_start(
                    rstd_dram[:, P * P :].rearrange("o (t p) -> (o t) p", t=NT - P),
                    rT1,
                )

            NCH = NPOS // GCH  # 36 chunks
            RBC = 4 * GCH      # rstd broadcast chunk columns
            with (
                tc.tile_pool(name="p3_rb", bufs=2) as rbpool,
                tc.tile_pool(name="p3_ot", bufs=2) as otpool,
                tc.tile_pool(name="p3_ps", bufs=4, space="PSUM") as p3psum,
            ):
                # scale zt by rstd in place (4x DVE), then stream wo^T @ zt
                for bc in range(NPOS // RBC):
                    sl = slice(bc * RBC, (bc + 1) * RBC)
                    rb_t = rbpool.tile([C, RBC], BF16, tag="rb")
                    nc.sync.dma_start(rb_t, bcast_part(rstd_dram[:, sl], C))
                    nc.vector.tensor_tensor(
                        zt[:, sl], zt[:, sl], rb_t, mybir.AluOpType.mult
                    )
                OCH = 4  # psum chunks per output DMA
                for og in range(NCH // OCH):
                    ot = otpool.tile([D, OCH, GCH], F32, tag="ot")
                    for oc in range(OCH):
                        ch = og * OCH + oc
                        sl = slice(ch * GCH, (ch + 1) * GCH)
                        pr = p3psum.tile([D, GCH], F32, tag="pr")
                        nc.tensor.matmul(
                            pr, lhsT=wo_sb, rhs=zt[:, sl], start=True, stop=True
                        )
                        if with_bias:
                            nc.vector.tensor_scalar_add(pr, pr, bo_sb)
                        eng = nc.vector if oc % 2 == 0 else nc.scalar
                        if eng is nc.vector:
                            nc.vector.tensor_tensor(
                                ot[:, oc, :], pr, gg[:, sl], mybir.AluOpType.mult
                            )
                        else:
                            # keep ACT useful: evacuate with scale=1 then Pool
                            # multiplies the gate in SBUF
                            po = otpool.tile([D, GCH], BF16, tag="po")
                            nc.scalar.copy(po, pr)
                            nc.gpsimd.tensor_tensor(
                                ot[:, oc, :], po, gg[:, sl], mybir.AluOpType.mult
                            )
                    nc.sync.dma_start(
                        out_d[:, og * OCH * GCH : (og + 1) * OCH * GCH],
                        ot,
                    )
            p3big.release()
            p2big.release()

    nc.compile()
    return nc


_CACHE = {}


def _get_nc(n, with_bias, with_mask):
    key = (n, with_bias, with_mask)
    if key not in _CACHE:
        _CACHE[key] = build_nc(n=n, with_bias=with_bias, with_mask=with_mask)
    return _CACHE[key]


def prepare_host(z, mask, norm_g, norm_b, norm_out_g, norm_out_b,
                 Wa, ba, Wb, bb, Wga, bga, Wgb, bgb, Wo, bo, Wgo, bgo, n=N_FULL):
    """Fold norm affines + centering into weights; pre-normalize z rows."""
    f = np.asarray
    z = f(z, dtype=np.float32)
    mask = f(mask, dtype=np.float32)
    g = f(norm_g, np.float32)
    b = f(norm_b, np.float32)
    go = f(norm_out_g, np.float32)
    bo_n = f(norm_out_b, np.float32)

    # LN(z) @ W_aff + bias = (z*rstd) @ Wcen + (b @ W + bias),
    # Wcen = (I - J/D)(g .* W)
    def fold(Wm, bias):
        Wm = f(Wm, np.float32)
        Wg = g[:, None] * Wm
        Wcen = Wg - np.mean(Wg, axis=0, keepdims=True)
        return Wcen, f(bias, np.float32) + b @ Wm

    Wa_, ba_ = fold(Wa, ba)
    Wga_, bga_ = fold(Wga, bga)
    Wb_, bb_ = fold(Wb, bb)
    Wgb_, bgb_ = fold(Wgb, bgb)
    Wgo_, bgo_ = fold(Wgo, bgo)
    Wo32 = f(Wo, np.float32)
    Wog = go[:, None] * Wo32
    Wo_ = Wog - np.mean(Wog, axis=0, keepdims=True)
    bo_ = f(bo, np.float32) + bo_n @ Wo32

    bf = ml_dtypes.bfloat16
    wva_h = np.concatenate([Wb_, Wa_], axis=1).astype(bf)
    wg_h = np.concatenate([Wgb_, Wga_], axis=1).astype(bf)
    wgo_h = Wgo_.astype(bf)
    wo_h = Wo_.astype(bf)
    bva_h = np.concatenate([bb_, ba_])[None, :].astype(np.float32)
    bg_h = np.concatenate([bgb_, bga_])[None, :].astype(np.float32)

    with_bias = bool(
        np.any(bva_h) or np.any(bg_h) or np.any(bgo_) or np.any(bo_)
    )
    with_mask = not bool(np.all(mask == 1.0))

    # host-side LN stats: rstd per row of z, folded into z itself
    zf = z[0].reshape(n * n, D)
    m = zf.mean(axis=1, keepdims=True)
    v = ((zf - m) ** 2).mean(axis=1, keepdims=True)
    r = 1.0 / np.sqrt(v + EPS)
    zsf = (zf * r).astype(np.float32)

    SH = n // W
    NT = SH * n // P
    in_maps = []
    for mi in range(W):
        rows = zsf[SH * n * mi : SH * n * (mi + 1)]  # [SH*n, D]
        zs_h = np.ascontiguousarray(rows.T).astype(bf)  # [D, SH*n]
        im = {
            "zs": zs_h,
            "wva": wva_h,
            "wg": wg_h,
            "wgo": wgo_h,
            "wo": wo_h,
        }
        if with_bias:
            im["bva"] = bva_h
            im["bg"] = bg_h
            im["bgo"] = bgo_[:, None].astype(np.float32)
            im["bo"] = bo_[:, None].astype(np.float32)
        if with_mask:
            msk = mask[0].reshape(n * n)[SH * n * mi : SH * n * (mi + 1)]
            im["mask_sh"] = np.ascontiguousarray(
                msk.reshape(NT, P).T
            ).astype(np.float32)
        in_maps.append(im)
    return in_maps, with_bias, with_mask


def unshard(results, n=N_FULL):
    """results: list of per-core out_d arrays [D, SH*n] -> [1, n, n, D]."""
    SH = n // W
    parts = []
    for mi in range(W):
        o = results[mi].reshape(D, SH, n)
        parts.append(o.transpose(1, 2, 0))
    return np.concatenate(parts, axis=0)[None]


def kernel(**inputs):
    n = inputs["z"].shape[1]
    in_maps, with_bias, with_mask = prepare_host(**inputs, n=n)
    nc = _get_nc(n, with_bias, with_mask)
    res = run_bass_kernel_spmd(nc, in_maps, list(range(W)))
    out = unshard([res.results[m]["out_d"] for m in range(W)], n=n)
    return out.astype(np.float32)


# revision 22
# speedup vs baseline: 1.0468x; 1.0118x over previous
"""Trainium2 Bass kernel for CustomTriangleMultiplicationOutgoing.

Reference computation (B=1, N=384, D=C=128):
    z_norm = LN(z) * g + b                        # over D
    left   = (z_norm@Wa + ba) * sigmoid(z_norm@Wga + bga) * mask
    right  = (z_norm@Wb + bb) * sigmoid(z_norm@Wgb + bgb) * mask
    z_out[i,j,c] = sum_k left[i,k,c] * right[j,k,c]
    z_out  = LN(z_out) * g_out + b_out            # over C
    out    = (z_out@Wo + bo) * sigmoid(z_norm@Wgo + bgo)

Host folds the input LN into the weights (LN(z)@W = (z*rstd)@Wcen) and
pre-normalizes z; the device does plain matmuls.

Sharding: phase 1 is row-sharded (48 i-rows per core).  The einsum is
CHANNEL-sharded: an AllToAll (chunked by k for overlap) exchanges
left+right so each core holds all (i,k) for its 16 channels, giving
near-full PE utilization (96-row i-blocks).  A second AllToAll brings
z_out back row-sharded with c on partitions.  Output-LN stats are
accumulated per-channel-shard in phase 2 (bf16 DVE adds) and combined
with a tiny ReduceScatter; rstd is applied to z_out BEFORE the final
projection (it commutes), so phase 3 is a single weight-stationary
streamed matmul wo^T @ zt with the out-gate applied in [d, ij] layout.

Layout trick: the phase-1 gating multiply reads its PSUM pair strided
(r-innermost) and writes CONTIGUOUS pair-interleaved [rb, s, c, r2]
blocks -- strided DVE *writes* cost ~7ns/el while strided reads are
free, so all engine-side transposes are folded into reads and the
A2A staging DMA sees 64-byte runs.
"""

import numpy as np
import ml_dtypes

import concourse.bass as bass
import concourse.mybir as mybir
import concourse.tile as tile
from concourse import bacc
from concourse.masks import make_identity
from concourse.bass_utils import run_bass_kernel_spmd

F32 = mybir.dt.float32
BF16 = mybir.dt.bfloat16
EPS = 1e-5

B = 1
N_FULL = 384
D = 128
C = 128
W = 8  # cores
P = 128


def bcast_part(ap, parts):
    """Broadcast a [1, ...] AP across `parts` partitions (partition step 0)."""
    return bass.AP(tensor=ap.tensor, offset=ap.offset, ap=[[0, parts]] + ap.ap[1:])


def build_nc(n=N_FULL, with_bias=False, with_mask=False):
    """Build the SPMD Bass program (same program on all 8 cores)."""
    assert n % P == 0 and n % W == 0
    SH = n // W          # rows of i per core (48)
    KC = n // P          # 128-wide chunks of k (3)
    NT = SH * KC         # 128-row tiles per core (144); tile t=(r,kc): t=r*KC+kc
    CL = C // W          # local channels per core in phase 2 (16)
    CH = CL // 2         # channels per backward-A2A half (8)
    RB = SH // 2         # row pairs (24)
    NIB = KC             # phase-2 i-blocks of 128 rows (3)
    GCH = 512            # go-stream / phase-3 psum chunk columns
    NPOS = SH * n        # positions per core (18432)

    nc = bacc.Bacc(None, num_devices=W)

    zs = nc.declare_dram_parameter("zs", [P, NPOS], BF16, isOutput=False)
    wva = nc.declare_dram_parameter("wva", [D, 2 * C], BF16, isOutput=False)
    wg = nc.declare_dram_parameter("wg", [D, 2 * C], BF16, isOutput=False)
    wgo = nc.declare_dram_parameter("wgo", [D, D], BF16, isOutput=False)
    wo = nc.declare_dram_parameter("wo", [C, D], BF16, isOutput=False)
    if with_bias:
        bva_p = nc.declare_dram_parameter("bva", [1, 2 * C], F32, isOutput=False)
        bg_p = nc.declare_dram_parameter("bg", [1, 2 * C], F32, isOutput=False)
        bgo_p = nc.declare_dram_parameter("bgo", [D, 1], F32, isOutput=False)
        bo_p = nc.declare_dram_parameter("bo", [D, 1], F32, isOutput=False)
    if with_mask:
        mask_sh = nc.declare_dram_parameter("mask_sh", [P, NT], F32, isOutput=False)
    out_d = nc.declare_dram_parameter("out_d", [D, NPOS], F32, isOutput=True)

    # forward A2A, one chunk per kc; block to dest g = [s, k, rb, cl, r2]
    lra2a = [
        nc.dram_tensor(f"lra2a_{kc}", [W, 2, P, RB, CL, 2], BF16) for kc in range(KC)
    ]
    ga2a = [
        nc.dram_tensor(f"ga2a_{kc}", [W, 2, P, RB, CL, 2], BF16) for kc in range(KC)
    ]
    # backward A2A, one chunk per c-half; block to dest g = [c_loc, i_loc, j]
    zoa2a = [nc.dram_tensor(f"zoa2a_{h}", [W, CH, SH, n], BF16) for h in range(2)]
    gza2a = [nc.dram_tensor(f"gza2a_{h}", [W, CH, SH, n], BF16) for h in range(2)]
    rstd_dram = nc.dram_tensor("rstd_dram", [1, NPOS], BF16)

    with tile.TileContext(nc) as tc:
        with tc.tile_pool(name="singles", bufs=1) as singles:
            wva_sb = singles.tile([D, 2 * C], BF16)
            nc.sync.dma_start(wva_sb, wva[:])
            wg_sb = singles.tile([D, 2 * C], BF16)
            nc.sync.dma_start(wg_sb, wg[:])
            wgo_sb = singles.tile([D, D], BF16)
            nc.sync.dma_start(wgo_sb, wgo[:])
            wo_sb = singles.tile([C, D], BF16)
            nc.sync.dma_start(wo_sb, wo[:])
            eps_sb = singles.tile([P, 1], F32)
            nc.vector.memset(eps_sb, EPS)
            ones_bf = singles.tile([P, 1], BF16)
            nc.vector.memset(ones_bf, 1.0)
            ident = singles.tile([P, P], BF16)
            make_identity(nc, ident)
            if with_bias:
                bva_sb = singles.tile([P, 2 * C], F32)
                nc.sync.dma_start(bva_sb, bcast_part(bva_p[:], P))
                bg_sb = singles.tile([P, 2 * C], F32)
                nc.sync.dma_start(bg_sb, bcast_part(bg_p[:], P))
                bgo_sb = singles.tile([D, 1], F32)
                nc.sync.dma_start(bgo_sb, bgo_p[:])
                bo_sb = singles.tile([D, 1], F32)
                nc.sync.dma_start(bo_sb, bo_p[:])
            if with_mask:
                mask_sb = singles.tile([P, NT], F32)
                nc.sync.dma_start(mask_sb, mask_sh[:])

            # out-gate, [d, ij] layout; even go-chunks hold sigmoid already,
            # odd chunks raw (sigmoid applied in-place during phase 2)
            gg = singles.tile([D, NPOS], BF16)
            gg_v = gg.rearrange("d (r k) -> d r k", r=SH)

            # ---------------- phase 1: projections, kc-major ----------------
            p1pool = tc.alloc_tile_pool(name="p1", bufs=1)
            zs_sb = p1pool.tile([P, NPOS], BF16)
            for ch in range(8):
                w8 = NPOS // 8
                nc.sync.dma_start(
                    zs_sb[:, ch * w8 : (ch + 1) * w8],
                    zs[:, ch * w8 : (ch + 1) * w8],
                )
            zs_v = zs_sb.rearrange("d (r k) -> d r k", r=SH)
            # pair-interleaved gated projections: [k, kc, rb, s(R,L), c, r2]
            lr_buf = p1pool.tile([P, KC, RB, 2, C, 2], BF16)

            odd_go = []  # go-chunks needing their sigmoid in phase 2
            with (
                tc.tile_pool(name="p1_sg", bufs=3) as sgpool,
                tc.tile_pool(name="p1_pv", bufs=2, space="PSUM") as pvpool,
                tc.tile_pool(name="p1_pg", bufs=2, space="PSUM") as pgpool,
                tc.tile_pool(name="p1_go", bufs=2, space="PSUM") as gopool,
            ):
                for kc in range(KC):
                    for rp in range(RB):
                        r0 = 2 * rp
                        ts = [(r0 + j) * KC + kc for j in range(2)]
                        pv = pvpool.tile([P, 2, 2 * C], F32, tag="pv")
                        pg = pgpool.tile([P, 2, 2 * C], F32, tag="pg")
                        for j in range(2):
                            lhsT = zs_v[:, r0 + j, kc * P : (kc + 1) * P]
                            nc.tensor.matmul(
                                pv[:, j, :], lhsT=lhsT, rhs=wva_sb,
                                start=True, stop=True,
                            )
                            nc.tensor.matmul(
                                pg[:, j, :], lhsT=lhsT, rhs=wg_sb,
                                start=True, stop=True,
                            )
                        if with_bias:
                            for j in range(2):
                                nc.vector.tensor_tensor(
                                    pv[:, j, :], pv[:, j, :], bva_sb,
                                    mybir.AluOpType.add,
                                )
                                nc.vector.tensor_tensor(
                                    pg[:, j, :], pg[:, j, :], bg_sb,
                                    mybir.AluOpType.add,
                                )
                        sg = sgpool.tile([P, 2, 2 * C], BF16, tag="sg")
                        nc.scalar.activation(
                            sg, pg, mybir.ActivationFunctionType.Sigmoid
                        )
                        if with_mask:
                            for j in range(2):
                                nc.gpsimd.tensor_scalar_mul(
                                    sg[:, j, :], sg[:, j, :],
                                    mask_sb[:, ts[j] : ts[j] + 1],
                                )
                        # gating: strided psum reads (r2 innermost), contiguous
                        # pair-interleaved write [s, c, r2]
                        nc.vector.tensor_tensor(
                            lr_buf[:, kc, rp],
                            pv.rearrange("k r (s c) -> k s c r", s=2),
                            sg.rearrange("k r (s c) -> k s c r", s=2),
                            mybir.AluOpType.mult,
                        )
                    # stage this kc chunk (64B runs) and kick its AllToAll
                    for s in range(2):
                        for g in range(W):
                            nc.sync.dma_start(
                                lra2a[kc][g, s],
                                lr_buf[:, kc, :, s, CL * g : CL * (g + 1), :],
                            )
                    nc.gpsimd.collective_compute(
                        "AllToAll",
                        mybir.AluOpType.bypass,
                        replica_groups=[list(range(W))],
                        ins=[lra2a[kc][:]],
                        outs=[ga2a[kc][:]],
                    )
                    # out-gate stream for this kc: wgo stationary, zs moving
                    for ch in range(SH // 4):
                        gps = gopool.tile([D, GCH], F32, tag="go")
                        rhs = zs_v[:, 4 * ch : 4 * ch + 4, kc * P : (kc + 1) * P]
                        nc.tensor.matmul(
                            gps, lhsT=wgo_sb, rhs=rhs, start=True, stop=True
                        )
                        if with_bias:
                            nc.vector.tensor_scalar_add(gps, gps, bgo_sb)
                        dst = gg_v[:, 4 * ch : 4 * ch + 4, kc * P : (kc + 1) * P]
                        if (kc * (SH // 4) + ch) % 3 == 0:
                            nc.scalar.activation(
                                dst, gps, mybir.ActivationFunctionType.Sigmoid
                            )
                        else:
                            nc.vector.tensor_copy(dst, gps)
                            odd_go.append((kc, ch))

            p1pool.release()  # zs_sb, lr_buf freed (staged to DRAM)

            # ---------------- phase 2: channel-sharded einsum ----------------
            p2big = tc.alloc_tile_pool(name="p2big", bufs=1)
            # z_out staging [i(128) x cl x j] + bf16 stats accumulators
            zo_sb = [
                p2big.tile([P, CL, n], BF16, name=f"zo_sb{ib}") for ib in range(NIB)
            ]

            lr_all = tc.alloc_tile_pool(name="lr_all", bufs=1)
            L_all = [
                lr_all.tile([P, W, RB, CL, 2], BF16, name=f"L_all{kc}")
                for kc in range(KC)
            ]
            R_all = [
                lr_all.tile([P, W, RB, CL, 2], BF16, name=f"R_all{kc}")
                for kc in range(KC)
            ]
            for kc in range(KC):
                nc.sync.dma_start(
                    L_all[kc],
                    ga2a[kc][:, 1].rearrange("g k rb cl r -> k g rb cl r"),
                )
                nc.sync.dma_start(
                    R_all[kc],
                    ga2a[kc][:, 0].rearrange("g k rb cl r -> k g rb cl r"),
                )

            with (
                tc.tile_pool(name="p2_lc", bufs=3) as lcpool,
                tc.tile_pool(name="p2_sq", bufs=3) as sqpool,
                tc.tile_pool(name="p2_ps", bufs=6, space="PSUM") as p2psum,
            ):
                # finish the out-gate sigmoids on the otherwise-idle ACT
                for kc, ch in odd_go:
                    dst = gg_v[:, 4 * ch : 4 * ch + 4, kc * P : (kc + 1) * P]
                    nc.scalar.activation(
                        dst, dst, mybir.ActivationFunctionType.Sigmoid
                    )
                for cl in range(CL):
                    # weights APs allow only ONE free dim: gather this
                    # channel's left operand contiguously (strided DVE
                    # reads are free, writes contiguous)
                    lc = lcpool.tile([P, KC, n], BF16, tag="lc")
                    for kc in range(KC):
                        eng = nc.vector if (cl * KC + kc) % 3 else nc.gpsimd
                        eng.tensor_copy(
                            lc[:, kc].rearrange("k (s rb r) -> k s rb r", s=W, r=2),
                            L_all[kc][:, :, :, cl, :],
                        )
                    for ib in range(NIB):
                        ps = p2psum.tile([P, n], F32, tag="p2")
                        for kc in range(KC):
                            nc.tensor.matmul(
                                ps,
                                lhsT=lc[:, kc, ib * P : (ib + 1) * P],
                                rhs=R_all[kc][:, :, :, cl, :],
                                start=(kc == 0),
                                stop=(kc == KC - 1),
                            )
                        zslc = zo_sb[ib][:, cl, :]
                        nc.vector.tensor_copy(zslc, ps)
                    # after each c-half: stage + backward AllToAll
                    if cl == CH - 1 or cl == CL - 1:
                        h = 0 if cl < CH else 1
                        c0 = h * CH
                        for g in range(W):
                            lo, hi = g * SH, (g + 1) * SH
                            while lo < hi:
                                ib_ = lo // P
                                seg = min(hi, (ib_ + 1) * P) - lo
                                nc.sync.dma_start(
                                    zoa2a[h][g]
                                    .rearrange("c i j -> i c j")[
                                        lo - g * SH : lo - g * SH + seg
                                    ],
                                    zo_sb[ib_][lo - ib_ * P : lo - ib_ * P + seg,
                                               c0 : c0 + CH, :],
                                )
                                lo += seg
                        nc.gpsimd.collective_compute(
                            "AllToAll",
                            mybir.AluOpType.bypass,
                            replica_groups=[list(range(W))],
                            ins=[zoa2a[h][:]],
                            outs=[gza2a[h][:]],
                        )

            lr_all.release()

            # ---------------- phase 3: rstd, z_out @ Wo, gate ----------------
            p3big = tc.alloc_tile_pool(name="p3big", bufs=1)
            zt = p3big.tile([C, NPOS], BF16)  # z_out, c on partitions
            for h in range(2):
                for src in range(W):
                    nc.sync.dma_start(
                        zt[CL * src + CH * h : CL * src + CH * (h + 1), :],
                        gza2a[h][src].rearrange("c i j -> c (i j)"),
                    )
            # output-LN stats on the receiver: per-tile 1-column matmuls
            # against a ones vector contract over c (the partition dim)
            stpool = tc.alloc_tile_pool(name="p3_st", bufs=1, space="PSUM")
            S_ps = stpool.tile([P, NT], F32)
            SQ_ps = stpool.tile([P, NT], F32)
            with tc.tile_pool(name="p3_sq", bufs=3) as sq3pool:
                for qg in range(NT // 4):
                    sqq = sq3pool.tile([P, 4 * P], BF16, tag="sqq")
                    zq = zt[:, 4 * P * qg : 4 * P * (qg + 1)]
                    nc.vector.tensor_tensor(sqq, zq, zq, mybir.AluOpType.mult)
                    for tl in range(4):
                        t3 = 4 * qg + tl
                        nc.tensor.matmul(
                            S_ps[:, t3 : t3 + 1],
                            lhsT=zt[:, t3 * P : (t3 + 1) * P],
                            rhs=ones_bf, start=True, stop=True,
                        )
                        nc.tensor.matmul(
                            SQ_ps[:, t3 : t3 + 1],
                            lhsT=sqq[:, tl * P : (tl + 1) * P],
                            rhs=ones_bf, start=True, stop=True,
                        )
            # rstd in [p, t3] tile-major
            mean = p3big.tile([P, NT], F32)
            nc.vector.tensor_scalar_mul(mean, S_ps, 1.0 / C)
            msq = p3big.tile([P, NT], F32)
            nc.vector.tensor_scalar_mul(msq, SQ_ps, 1.0 / C)
            var = p3big.tile([P, NT], F32)
            nc.vector.tensor_tensor(var, mean, mean, mybir.AluOpType.mult)
            nc.vector.tensor_tensor(var, msq, var, mybir.AluOpType.subtract)
            rstd = p3big.tile([P, NT], F32)
            nc.scalar.activation(
                rstd, var, mybir.ActivationFunctionType.Sqrt, bias=eps_sb
            )
            nc.vector.reciprocal(rstd, rstd)
            rstd_bf = p3big.tile([P, NT], BF16)
            nc.vector.tensor_copy(rstd_bf, rstd)
            stpool.release()
            # transpose [p, t3] -> position-major row in DRAM (PE transpose,
            # then contiguous DMA; replicated DMA reads broadcast it back)
            with tc.tile_pool(name="p3_tp", bufs=2, space="PSUM") as tppool:
                tp0 = tppool.tile([P, P], BF16, tag="tp0")
                nc.tensor.transpose(tp0, rstd_bf[:, 0:P], ident)
                rT0 = p3big.tile([P, P], BF16)
                nc.vector.tensor_copy(rT0, tp0)
                nc.sync.dma_start(
                    rstd_dram[:, 0 : P * P].rearrange("o (t p) -> (o t) p", t=P),
                    rT0,
                )
                tp1 = tppool.tile([NT - P, P], BF16, tag="tp1")
                nc.tensor.transpose(tp1, rstd_bf[:, P:NT], ident)
                rT1 = p3big.tile([NT - P, P], BF16)
                nc.vector.tensor_copy(rT1, tp1)
                nc.sync.dma_start(
                    rstd_dram[:, P * P :].rearrange("o (t p) -> (o t) p", t=NT - P),
                    rT1,
                )

            NCH = NPOS // GCH  # 36 chunks
            RBC = 4 * GCH      # rstd broadcast chunk columns
            with (
                tc.tile_pool(name="p3_rb", bufs=2) as rbpool,
                tc.tile_pool(name="p3_ot", bufs=2) as otpool,
                tc.tile_pool(name="p3_ps", bufs=4, space="PSUM") as p3psum,
            ):
                # scale zt by rstd in place (4x DVE), then stream wo^T @ zt
                for bc in range(NPOS // RBC):
                    sl = slice(bc * RBC, (bc + 1) * RBC)
                    rb_t = rbpool.tile([C, RBC], BF16, tag="rb")
                    nc.sync.dma_start(rb_t, bcast_part(rstd_dram[:, sl], C))
                    nc.vector.tensor_tensor(
                        zt[:, sl], zt[:, sl], rb_t, mybir.AluOpType.mult
                    )
                OCH = 4  # psum chunks per output DMA
                for og in range(NCH // OCH):
                    ot = otpool.tile([D, OCH, GCH], F32, tag="ot")
                    for oc in range(OCH):
                        ch = og * OCH + oc
                        sl = slice(ch * GCH, (ch + 1) * GCH)
                        pr = p3psum.tile([D, GCH], F32, tag="pr")
                        nc.tensor.matmul(
                            pr, lhsT=wo_sb, rhs=zt[:, sl], start=True, stop=True
                        )
                        if with_bias:
                            nc.vector.tensor_scalar_add(pr, pr, bo_sb)
                        nc.vector.tensor_tensor(
                            ot[:, oc, :], pr, gg[:, sl], mybir.AluOpType.mult
                        )
                    nc.sync.dma_start(
                        out_d[:, og * OCH * GCH : (og + 1) * OCH * GCH],
                        ot,
                    )
            p3big.release()
            p2big.release()

    nc.compile()
    return nc


_CACHE = {}


def _get_nc(n, with_bias, with_mask):
    key = (n, with_bias, with_mask)
    if key not in _CACHE:
        _CACHE[key] = build_nc(n=n, with_bias=with_bias, with_mask=with_mask)
    return _CACHE[key]


def prepare_host(z, mask, norm_g, norm_b, norm_out_g, norm_out_b,
                 Wa, ba, Wb, bb, Wga, bga, Wgb, bgb, Wo, bo, Wgo, bgo, n=N_FULL):
    """Fold norm affines + centering into weights; pre-normalize z rows."""
    f = np.asarray
    z = f(z, dtype=np.float32)
    mask = f(mask, dtype=np.float32)
    g = f(norm_g, np.float32)
    b = f(norm_b, np.float32)
    go = f(norm_out_g, np.float32)
    bo_n = f(norm_out_b, np.float32)

    # LN(z) @ W_aff + bias = (z*rstd) @ Wcen + (b @ W + bias),
    # Wcen = (I - J/D)(g .* W)
    def fold(Wm, bias):
        Wm = f(Wm, np.float32)
        Wg = g[:, None] * Wm
        Wcen = Wg - np.mean(Wg, axis=0, keepdims=True)
        return Wcen, f(bias, np.float32) + b @ Wm

    Wa_, ba_ = fold(Wa, ba)
    Wga_, bga_ = fold(Wga, bga)
    Wb_, bb_ = fold(Wb, bb)
    Wgb_, bgb_ = fold(Wgb, bgb)
    Wgo_, bgo_ = fold(Wgo, bgo)
    Wo32 = f(Wo, np.float32)
    Wog = go[:, None] * Wo32
    Wo_ = Wog - np.mean(Wog, axis=0, keepdims=True)
    bo_ = f(bo, np.float32) + bo_n @ Wo32

    bf = ml_dtypes.bfloat16
    wva_h = np.concatenate([Wb_, Wa_], axis=1).astype(bf)
    wg_h = np.concatenate([Wgb_, Wga_], axis=1).astype(bf)
    wgo_h = Wgo_.astype(bf)
    wo_h = Wo_.astype(bf)
    bva_h = np.concatenate([bb_, ba_])[None, :].astype(np.float32)
    bg_h = np.concatenate([bgb_, bga_])[None, :].astype(np.float32)

    with_bias = bool(
        np.any(bva_h) or np.any(bg_h) or np.any(bgo_) or np.any(bo_)
    )
    with_mask = not bool(np.all(mask == 1.0))

    # host-side LN stats: rstd per row of z, folded into z itself
    zf = z[0].reshape(n * n, D)
    m = zf.mean(axis=1, keepdims=True)
    v = ((zf - m) ** 2).mean(axis=1, keepdims=True)
    r = 1.0 / np.sqrt(v + EPS)
    zsf = (zf * r).astype(np.float32)

    SH = n // W
    NT = SH * n // P
    in_maps = []
    for mi in range(W):
        rows = zsf[SH * n * mi : SH * n * (mi + 1)]  # [SH*n, D]
        zs_h = np.ascontiguousarray(rows.T).astype(bf)  # [D, SH*n]
        im = {
            "zs": zs_h,
            "wva": wva_h,
            "wg": wg_h,
            "wgo": wgo_h,
            "wo": wo_h,
        }
        if with_bias:
            im["bva"] = bva_h
            im["bg"] = bg_h
            im["bgo"] = bgo_[:, None].astype(np.float32)
            im["bo"] = bo_[:, None].astype(np.float32)
        if with_mask:
            msk = mask[0].reshape(n * n)[SH * n * mi : SH * n * (mi + 1)]
            im["mask_sh"] = np.ascontiguousarray(
                msk.reshape(NT, P).T
            ).astype(np.float32)
        in_maps.append(im)
    return in_maps, with_bias, with_mask


def unshard(results, n=N_FULL):
    """results: list of per-core out_d arrays [D, SH*n] -> [1, n, n, D]."""
    SH = n // W
    parts = []
    for mi in range(W):
        o = results[mi].reshape(D, SH, n)
        parts.append(o.transpose(1, 2, 0))
    return np.concatenate(parts, axis=0)[None]


def kernel(**inputs):
    n = inputs["z"].shape[1]
    in_maps, with_bias, with_mask = prepare_host(**inputs, n=n)
    nc = _get_nc(n, with_bias, with_mask)
    res = run_bass_kernel_spmd(nc, in_maps, list(range(W)))
    out = unshard([res.results[m]["out_d"] for m in range(W)], n=n)
    return out.astype(np.float32)
